# revision 13
# baseline (speedup 1.0000x reference)
"""Trainium2 Bass kernel for the ACSL multi-snippet classification loss.

Algorithm (derived from the reference):
  loss = sum_{i,c} wm_last[i,c] * cls_loss[i,c] / (n_i*T)
  cls_loss[i,c] = sum_t softplus(lg[i,c,t]) - sum_t [c == argmax_c' lb[i,c',t]] * lg[i,c,t]
  wm_last depends only on snippet t=99 plus fixed (input-independent) jax randomness.

Device does the O(N) work (reads both full tensors once):
  - sp_sum[i,c] = sum_t softplus(lg[i,c,t])           (ScalarE softplus + DVE reduce)
  - keymax[i,t] = max_c ( fp16(lb*K) + (200-c)*2^-25 )  packed value+index argmax
    (ScalarE fp16 quantize, GpSimd index-pack add, DVE reduce)
Host does the tiny [1024,201]-scale finalization: index extraction from keymax,
argmax-gather subtraction, last-snippet weight mask, final weighted sum.

Sharding: data-parallel over rows (n_i axis), 128 rows per core across 8 cores.
"""

import numpy as np

N_ROWS = 1024
N_C = 201
NUM_CLASSES = 200
T = 100
N_CORES = 8
P = N_ROWS // N_CORES  # 128 rows per core == SBUF partitions
SCORE_THR = 0.3
# 0.5*(1-2^-11): guarantees fp16(lb*K) < 0.5 strictly, so key = qlb + (200-c)*2^-25
# is exactly representable in f32 and the index is exactly recoverable on host.
PACK_SCALE = 0.499755859375
IDX_LSB = 2.0 ** -25
CHUNKS = [29, 29, 29, 29, 29, 29, 27]  # class-axis chunks for the logits side
T_CHUNKS = [25, 25, 25, 25]  # time-axis chunks for the (host-transposed) labels side
# engine for the fp16 quantize of each labels chunk: balance DVE vs GpSimd
QUANT_ON_VECTOR = [True, False, False, False]

_CACHE = {}


def _patch_act_tables():
    """Prefer the table set containing BOTH exp and ln so the per-chunk
    Exp->Ln sequence needs one ACT_TABLE_LOAD total instead of 14."""
    from concourse import bacc as bacc_mod

    orig = bacc_mod.get_activation_tables
    if getattr(orig, "_patched_for_ln_exp", False):
        return

    def patched(arch):
        # Dict order IS the act_func_set_id wired into the NEFF, so it must
        # not change. Instead remove Exp/Ln from every other set so the
        # table chooser can only satisfy them from the combined set.
        from concourse import mybir

        t = dict(orig(arch))
        pref = "natural_log_exp_and_others"
        if pref in t:
            both = {
                mybir.ActivationFunctionType.Exp,
                mybir.ActivationFunctionType.Ln,
            }
            t = {
                k: (v if k == pref else set(v) - both) for k, v in t.items()
            }
        return t

    patched._patched_for_ln_exp = True
    bacc_mod.get_activation_tables = patched


def _build():
    """Build + compile the per-core Bass program (same SPMD program on all 8)."""
    from contextlib import ExitStack
    from concourse import bacc, mybir, tile

    _patch_act_tables()
    nc = bacc.Bacc(
        "TRN2", target_bir_lowering=False, debug=False, num_devices=N_CORES
    )
    f32 = mybir.dt.float32
    f16 = mybir.dt.float16
    AF = mybir.ActivationFunctionType
    ALU = mybir.AluOpType
    AX = mybir.AxisListType

    lg_ext = nc.dram_tensor("lg", [P, N_C, T], f32, kind="ExternalInput").ap()
    # labels arrive host-transposed to [P, T, N_C] so every class-axis op on
    # the device reads/writes with a contiguous inner axis
    lb_ext = nc.dram_tensor("lb", [P, T, N_C], f32, kind="ExternalInput").ap()
    ik_ext = nc.dram_tensor("idxk", [P, N_C], f32, kind="ExternalInput").ap()
    out_ext = nc.dram_tensor("out", [P, N_C + T], f32, kind="ExternalOutput").ap()

    with tile.TileContext(nc) as tc, ExitStack() as ctx:
        const_pool = ctx.enter_context(tc.tile_pool(name="const", bufs=1))
        lb_pool = ctx.enter_context(tc.tile_pool(name="lbp", bufs=2))
        lg_pool = ctx.enter_context(tc.tile_pool(name="lgp", bufs=2))
        e_pool = ctx.enter_context(tc.tile_pool(name="ep", bufs=2))
        sp_pool = ctx.enter_context(tc.tile_pool(name="spp", bufs=2))
        q_pool = ctx.enter_context(tc.tile_pool(name="qp", bufs=2))
        key_pool = ctx.enter_context(tc.tile_pool(name="keyp", bufs=2))
        acc_pool = ctx.enter_context(tc.tile_pool(name="accp", bufs=1))

        ik = const_pool.tile([P, N_C], f32)
        nc.sync.dma_start(out=ik[:], in_=ik_ext[:])

        sp_out = acc_pool.tile([P, N_C], f32)
        keymax = acc_pool.tile([P, T], f32)

        # ---- logits side: softplus + sum over t (natural [c][t] layout) ----
        c0 = 0
        for cc in CHUNKS:
            n = cc * T
            tlg = lg_pool.tile([P, n], f32, tag="lg")
            nc.sync.dma_start(
                out=tlg[:].rearrange("p (c t) -> p c t", t=T),
                in_=lg_ext[:, c0 : c0 + cc, :],
            )
            # softplus = Ln(Exp(x) + 1): both funcs in one ACT table set
            te = e_pool.tile([P, n], f32, tag="e")
            nc.scalar.activation(te[:], tlg[:], AF.Exp)
            tsp = sp_pool.tile([P, n], f32, tag="sp")
            nc.scalar.activation(tsp[:], te[:], AF.Ln, bias=1.0)
            nc.vector.tensor_reduce(
                out=sp_out[:, c0 : c0 + cc],
                in_=tsp[:].rearrange("p (c t) -> p c t", t=T),
                axis=AX.X,
                op=ALU.add,
            )
            c0 += cc

        # ---- labels side: quantize + index-pack + max over c ([t][c] layout) ----
        t0 = 0
        for ti, tc_sz in enumerate(T_CHUNKS):
            n = tc_sz * N_C
            tlb = lb_pool.tile([P, n], f32, tag="lb")
            nc.sync.dma_start(
                out=tlb[:].rearrange("p (t c) -> p t c", c=N_C),
                in_=lb_ext[:, t0 : t0 + tc_sz, :],
            )
            tq = q_pool.tile([P, n], f16, tag="q")
            qeng = nc.vector if QUANT_ON_VECTOR[ti] else nc.gpsimd
            qeng.tensor_scalar_mul(tq[:], tlb[:], PACK_SCALE)
            tkey = key_pool.tile([P, n], f32, tag="key")
            nc.gpsimd.tensor_tensor(
                out=tkey[:].rearrange("p (t c) -> p t c", c=N_C),
                in0=tq[:].rearrange("p (t c) -> p t c", c=N_C),
                in1=ik[:].unsqueeze(1).broadcast_to([P, tc_sz, N_C]),
                op=ALU.add,
            )
            nc.vector.tensor_reduce(
                out=keymax[:, t0 : t0 + tc_sz],
                in_=tkey[:].rearrange("p (t c) -> p t c", c=N_C),
                axis=AX.X,
                op=ALU.max,
            )
            t0 += tc_sz

        nc.sync.dma_start(out=out_ext[:, 0:N_C], in_=sp_out[:])
        nc.sync.dma_start(out=out_ext[:, N_C : N_C + T], in_=keymax[:])

    nc.compile()
    return nc


def _get_nc():
    if "nc" not in _CACHE:
        _CACHE["nc"] = _build()
    return _CACHE["nc"]


def run_device(lg, lb, trace=False, **kw):
    """Run the SPMD device program. Returns (sp_sum[1024,201], keymax[1024,100], results)."""
    from concourse.bass_utils import run_bass_kernel_spmd

    nc = _get_nc()
    idxk = ((NUM_CLASSES - np.arange(N_C)) * IDX_LSB).astype(np.float32)
    ik_tile = np.ascontiguousarray(np.broadcast_to(idxk, (P, N_C)))
    lbT = np.ascontiguousarray(lb.transpose(0, 2, 1))  # [rows, T, N_C]
    in_maps = []
    for core in range(N_CORES):
        r0 = core * P
        in_maps.append(
            {
                "lg": np.ascontiguousarray(lg[r0 : r0 + P]),
                "lb": lbT[r0 : r0 + P],
                "idxk": ik_tile,
            }
        )
    res = run_bass_kernel_spmd(
        nc, in_maps, core_ids=list(range(N_CORES)), trace=trace, **kw
    )
    out_full = np.concatenate(
        [np.asarray(res.results[i]["out"]) for i in range(N_CORES)], axis=0
    )
    return out_full[:, :N_C], out_full[:, N_C:], res


def _host_finalize(lg, lb, sp_sum, keymax):
    """Tiny [1024,201]-scale finalization mirroring the reference semantics."""
    import jax
    import jax.numpy as jnp

    # --- extract per-(i,t) argmax class from the packed keymax ---
    qlb_rec = keymax.astype(np.float16).astype(np.float32)
    cprime = np.rint(
        (keymax.astype(np.float64) - qlb_rec.astype(np.float64)) / IDX_LSB
    ).astype(np.int64)
    idx = NUM_CLASSES - cprime
    np.clip(idx, 0, NUM_CLASSES, out=idx)  # safety; always in range in practice

    # --- cls_loss = sp_sum - scatter-subtract of gathered logits ---
    ii = np.arange(N_ROWS)[:, None]
    tt = np.arange(T)[None, :]
    g = lg[ii, idx, tt].astype(np.float64)
    cls_loss = sp_sum.astype(np.float64).copy()
    np.add.at(cls_loss, (ii, idx), -g)

    # --- last-snippet weight mask (exact reference semantics) ---
    lg99 = lg[:, :, T - 1]
    lb99 = lb[:, :, T - 1]
    labels99 = lb99.argmax(axis=1)
    is_bg = labels99 == NUM_CLASSES
    n_bg = int(is_bg.sum())

    cpu = jax.devices("cpu")[0]
    with jax.default_device(cpu):
        keys = jax.random.split(jax.random.key(42), T)
        k1, k2 = jax.random.split(keys[T - 1])
        u1 = np.asarray(jax.random.uniform(k1, (N_ROWS,)))
        u2 = np.asarray(jax.random.uniform(k2, (N_ROWS,)))
        score_mask = np.asarray(jax.nn.sigmoid(jnp.asarray(lg99))) >= np.float32(
            SCORE_THR
        )

    def _sel(u, m):
        um = np.where(is_bg, u, np.inf).astype(np.float32)
        order = np.argsort(um, kind="stable")
        ranks = np.zeros(N_ROWS, np.int64)
        ranks[order] = np.arange(N_ROWS)
        return is_bg & (ranks < m)

    sel_rare = _sel(u1, n_bg // 100)
    sel_common = _sel(u2, n_bg // 10)

    cls_id = np.arange(N_C)
    rare_m = (cls_id < 50).astype(np.float64)
    common_m = ((cls_id >= 50) & (cls_id < 150)).astype(np.float64)
    freq_m = ((cls_id >= 150) & (cls_id < 200)).astype(np.float64)
    bg_col = (cls_id == NUM_CLASSES).astype(np.float64)

    target99 = (labels99[:, None] == cls_id[None, :]).astype(np.float64)
    wm = np.where(is_bg[:, None], 0.0, score_mask.astype(np.float64))
    ind = (
        target99
        + is_bg[:, None] * (freq_m + bg_col)[None, :]
        + sel_rare[:, None] * rare_m[None, :]
        + sel_common[:, None] * common_m[None, :]
    )
    wm = np.maximum(wm, np.clip(ind, 0.0, 1.0))

    loss = (wm * cls_loss).sum() / (N_ROWS * T)
    return np.array(loss, dtype=np.float32)


def kernel(cls_logits_, labels_):
    lg = np.ascontiguousarray(np.asarray(cls_logits_, dtype=np.float32))
    lb = np.ascontiguousarray(np.asarray(labels_, dtype=np.float32))
    sp_sum, keymax, _ = run_device(lg, lb, trace=False)
    return _host_finalize(lg, lb, sp_sum, keymax)


# revision 14
# speedup vs baseline: 2.7834x; 2.7834x over previous
"""Trainium2 Bass kernel for the ACSL multi-snippet classification loss.

Algorithm (derived from the reference):
  loss = sum_{i,c} wm_last[i,c] * cls_loss[i,c] / (n_i*T)
  cls_loss[i,c] = sum_t softplus(lg[i,c,t]) - sum_t [c == argmax_c' lb[i,c',t]] * lg[i,c,t]
  wm_last depends only on snippet t=99 plus fixed (input-independent) jax randomness.

Device does the O(N) work (reads both full tensors once):
  - sp_sum[i,c] = sum_t softplus(lg[i,c,t])           (ScalarE softplus + DVE reduce)
  - keymax[i,t] = max_c ( fp16(lb*K) + (200-c)*2^-25 )  packed value+index argmax
    (ScalarE fp16 quantize, GpSimd index-pack add, DVE reduce)
Host does the tiny [1024,201]-scale finalization: index extraction from keymax,
argmax-gather subtraction, last-snippet weight mask, final weighted sum.

Sharding: data-parallel over rows (n_i axis), 128 rows per core across 8 cores.
"""

import numpy as np

N_ROWS = 1024
N_C = 201
NUM_CLASSES = 200
T = 100
N_CORES = 8
P = N_ROWS // N_CORES  # 128 rows per core == SBUF partitions
SCORE_THR = 0.3
# 0.5*(1-2^-11): guarantees fp16(lb*K) < 0.5 strictly, so key = qlb + (200-c)*2^-25
# is exactly representable in f32 and the index is exactly recoverable on host.
PACK_SCALE = 0.499755859375
IDX_LSB = 2.0 ** -25
CHUNKS = [29, 29, 29, 29, 29, 29, 27]  # class-axis chunks for the logits side
T_CHUNKS = [25, 25, 25, 25]  # time-axis chunks for the (host-transposed) labels side
# engine for the fp16 quantize of each labels chunk: GpSimd tensor_scalar with
# fp16 output measured pathologically slow (17 cyc/elem), so keep all on DVE
QUANT_ON_VECTOR = [True, True, True, True]

_CACHE = {}


def _patch_act_tables():
    """Prefer the table set containing BOTH exp and ln so the per-chunk
    Exp->Ln sequence needs one ACT_TABLE_LOAD total instead of 14."""
    from concourse import bacc as bacc_mod

    orig = bacc_mod.get_activation_tables
    if getattr(orig, "_patched_for_ln_exp", False):
        return

    def patched(arch):
        # Dict order IS the act_func_set_id wired into the NEFF, so it must
        # not change. Instead remove Exp/Ln from every other set so the
        # table chooser can only satisfy them from the combined set.
        from concourse import mybir

        t = dict(orig(arch))
        pref = "natural_log_exp_and_others"
        if pref in t:
            both = {
                mybir.ActivationFunctionType.Exp,
                mybir.ActivationFunctionType.Ln,
            }
            t = {
                k: (v if k == pref else set(v) - both) for k, v in t.items()
            }
        return t

    patched._patched_for_ln_exp = True
    bacc_mod.get_activation_tables = patched


def _build():
    """Build + compile the per-core Bass program (same SPMD program on all 8)."""
    from contextlib import ExitStack
    from concourse import bacc, mybir, tile

    _patch_act_tables()
    nc = bacc.Bacc(
        "TRN2", target_bir_lowering=False, debug=False, num_devices=N_CORES
    )
    f32 = mybir.dt.float32
    f16 = mybir.dt.float16
    AF = mybir.ActivationFunctionType
    ALU = mybir.AluOpType
    AX = mybir.AxisListType

    lg_ext = nc.dram_tensor("lg", [P, N_C, T], f32, kind="ExternalInput").ap()
    # labels arrive host-transposed to [P, T, N_C] so every class-axis op on
    # the device reads/writes with a contiguous inner axis
    lb_ext = nc.dram_tensor("lb", [P, T, N_C], f32, kind="ExternalInput").ap()
    ik_ext = nc.dram_tensor("idxk", [P, N_C], f32, kind="ExternalInput").ap()
    out_ext = nc.dram_tensor("out", [P, N_C + T], f32, kind="ExternalOutput").ap()

    with tile.TileContext(nc) as tc, ExitStack() as ctx:
        const_pool = ctx.enter_context(tc.tile_pool(name="const", bufs=1))
        lb_pool = ctx.enter_context(tc.tile_pool(name="lbp", bufs=2))
        lg_pool = ctx.enter_context(tc.tile_pool(name="lgp", bufs=2))
        e_pool = ctx.enter_context(tc.tile_pool(name="ep", bufs=2))
        sp_pool = ctx.enter_context(tc.tile_pool(name="spp", bufs=2))
        q_pool = ctx.enter_context(tc.tile_pool(name="qp", bufs=2))
        key_pool = ctx.enter_context(tc.tile_pool(name="keyp", bufs=2))
        acc_pool = ctx.enter_context(tc.tile_pool(name="accp", bufs=1))

        ik = const_pool.tile([P, N_C], f32)
        nc.sync.dma_start(out=ik[:], in_=ik_ext[:])

        sp_out = acc_pool.tile([P, N_C], f32)
        keymax = acc_pool.tile([P, T], f32)

        # ---- logits side: softplus + sum over t (natural [c][t] layout) ----
        c0 = 0
        for cc in CHUNKS:
            n = cc * T
            tlg = lg_pool.tile([P, n], f32, tag="lg")
            nc.sync.dma_start(
                out=tlg[:].rearrange("p (c t) -> p c t", t=T),
                in_=lg_ext[:, c0 : c0 + cc, :],
            )
            # softplus = Ln(Exp(x) + 1): both funcs in one ACT table set
            te = e_pool.tile([P, n], f32, tag="e")
            nc.scalar.activation(te[:], tlg[:], AF.Exp)
            tsp = sp_pool.tile([P, n], f32, tag="sp")
            nc.scalar.activation(tsp[:], te[:], AF.Ln, bias=1.0)
            nc.vector.tensor_reduce(
                out=sp_out[:, c0 : c0 + cc],
                in_=tsp[:].rearrange("p (c t) -> p c t", t=T),
                axis=AX.X,
                op=ALU.add,
            )
            c0 += cc

        # ---- labels side: quantize + index-pack + max over c ([t][c] layout) ----
        t0 = 0
        for ti, tc_sz in enumerate(T_CHUNKS):
            n = tc_sz * N_C
            tlb = lb_pool.tile([P, n], f32, tag="lb")
            nc.sync.dma_start(
                out=tlb[:].rearrange("p (t c) -> p t c", c=N_C),
                in_=lb_ext[:, t0 : t0 + tc_sz, :],
            )
            tq = q_pool.tile([P, n], f16, tag="q")
            qeng = nc.vector if QUANT_ON_VECTOR[ti] else nc.gpsimd
            qeng.tensor_scalar_mul(tq[:], tlb[:], PACK_SCALE)
            tkey = key_pool.tile([P, n], f32, tag="key")
            nc.gpsimd.tensor_tensor(
                out=tkey[:].rearrange("p (t c) -> p t c", c=N_C),
                in0=tq[:].rearrange("p (t c) -> p t c", c=N_C),
                in1=ik[:].unsqueeze(1).broadcast_to([P, tc_sz, N_C]),
                op=ALU.add,
            )
            nc.vector.tensor_reduce(
                out=keymax[:, t0 : t0 + tc_sz],
                in_=tkey[:].rearrange("p (t c) -> p t c", c=N_C),
                axis=AX.X,
                op=ALU.max,
            )
            t0 += tc_sz

        nc.sync.dma_start(out=out_ext[:, 0:N_C], in_=sp_out[:])
        nc.sync.dma_start(out=out_ext[:, N_C : N_C + T], in_=keymax[:])

    nc.compile()
    return nc


def _get_nc():
    if "nc" not in _CACHE:
        _CACHE["nc"] = _build()
    return _CACHE["nc"]


def run_device(lg, lb, trace=False, **kw):
    """Run the SPMD device program. Returns (sp_sum[1024,201], keymax[1024,100], results)."""
    from concourse.bass_utils import run_bass_kernel_spmd

    nc = _get_nc()
    idxk = ((NUM_CLASSES - np.arange(N_C)) * IDX_LSB).astype(np.float32)
    ik_tile = np.ascontiguousarray(np.broadcast_to(idxk, (P, N_C)))
    lbT = np.ascontiguousarray(lb.transpose(0, 2, 1))  # [rows, T, N_C]
    in_maps = []
    for core in range(N_CORES):
        r0 = core * P
        in_maps.append(
            {
                "lg": np.ascontiguousarray(lg[r0 : r0 + P]),
                "lb": lbT[r0 : r0 + P],
                "idxk": ik_tile,
            }
        )
    res = run_bass_kernel_spmd(
        nc, in_maps, core_ids=list(range(N_CORES)), trace=trace, **kw
    )
    out_full = np.concatenate(
        [np.asarray(res.results[i]["out"]) for i in range(N_CORES)], axis=0
    )
    return out_full[:, :N_C], out_full[:, N_C:], res


def _host_finalize(lg, lb, sp_sum, keymax):
    """Tiny [1024,201]-scale finalization mirroring the reference semantics."""
    import jax
    import jax.numpy as jnp

    # --- extract per-(i,t) argmax class from the packed keymax ---
    qlb_rec = keymax.astype(np.float16).astype(np.float32)
    cprime = np.rint(
        (keymax.astype(np.float64) - qlb_rec.astype(np.float64)) / IDX_LSB
    ).astype(np.int64)
    idx = NUM_CLASSES - cprime
    np.clip(idx, 0, NUM_CLASSES, out=idx)  # safety; always in range in practice

    # --- cls_loss = sp_sum - scatter-subtract of gathered logits ---
    ii = np.arange(N_ROWS)[:, None]
    tt = np.arange(T)[None, :]
    g = lg[ii, idx, tt].astype(np.float64)
    cls_loss = sp_sum.astype(np.float64).copy()
    np.add.at(cls_loss, (ii, idx), -g)

    # --- last-snippet weight mask (exact reference semantics) ---
    lg99 = lg[:, :, T - 1]
    lb99 = lb[:, :, T - 1]
    labels99 = lb99.argmax(axis=1)
    is_bg = labels99 == NUM_CLASSES
    n_bg = int(is_bg.sum())

    cpu = jax.devices("cpu")[0]
    with jax.default_device(cpu):
        keys = jax.random.split(jax.random.key(42), T)
        k1, k2 = jax.random.split(keys[T - 1])
        u1 = np.asarray(jax.random.uniform(k1, (N_ROWS,)))
        u2 = np.asarray(jax.random.uniform(k2, (N_ROWS,)))
        score_mask = np.asarray(jax.nn.sigmoid(jnp.asarray(lg99))) >= np.float32(
            SCORE_THR
        )

    def _sel(u, m):
        um = np.where(is_bg, u, np.inf).astype(np.float32)
        order = np.argsort(um, kind="stable")
        ranks = np.zeros(N_ROWS, np.int64)
        ranks[order] = np.arange(N_ROWS)
        return is_bg & (ranks < m)

    sel_rare = _sel(u1, n_bg // 100)
    sel_common = _sel(u2, n_bg // 10)

    cls_id = np.arange(N_C)
    rare_m = (cls_id < 50).astype(np.float64)
    common_m = ((cls_id >= 50) & (cls_id < 150)).astype(np.float64)
    freq_m = ((cls_id >= 150) & (cls_id < 200)).astype(np.float64)
    bg_col = (cls_id == NUM_CLASSES).astype(np.float64)

    target99 = (labels99[:, None] == cls_id[None, :]).astype(np.float64)
    wm = np.where(is_bg[:, None], 0.0, score_mask.astype(np.float64))
    ind = (
        target99
        + is_bg[:, None] * (freq_m + bg_col)[None, :]
        + sel_rare[:, None] * rare_m[None, :]
        + sel_common[:, None] * common_m[None, :]
    )
    wm = np.maximum(wm, np.clip(ind, 0.0, 1.0))

    loss = (wm * cls_loss).sum() / (N_ROWS * T)
    return np.array(loss, dtype=np.float32)


def kernel(cls_logits_, labels_):
    lg = np.ascontiguousarray(np.asarray(cls_logits_, dtype=np.float32))
    lb = np.ascontiguousarray(np.asarray(labels_, dtype=np.float32))
    sp_sum, keymax, _ = run_device(lg, lb, trace=False)
    return _host_finalize(lg, lb, sp_sum, keymax)
